# revision 1
# baseline (speedup 1.0000x reference)
"""Trainium2 Bass kernel for nn_Decoder (LSTM over T steps + final FC).

Problem: y_hist [256, 512], LSTM(input_size=1, hidden=1024), h0/c0 [256, 1024],
output = h_T @ W_fc.T + b_fc -> [256, 1].

Sharding: data-parallel. Batch 256 -> 8 cores x 32 rows. LSTM + fc weights
replicated on every core; the time recurrence stays local per core (no
collectives).

Per-core per-step compute (batch-in-partitions orientation, column-tiled):
  gates[32, 4096] = h[32,1024] @ W_hh^T  (+ x_t * w_in + bias)
  - The 128x128 PE array runs 4 concurrent M=32 matmuls via column tiling
    (tile_position=(0, 32q)); operands bf16 (walrus rejects col-tiled
    fp32/f32r), accumulation fp32 in PSUM. Cell state c stays fp32.
  - PE column group q (PSUM partitions 32q:32q+32) computes all four gates
    for H-quarter q. PSUM PS [128, 1024]: cols [0:256)=i, [256:512)=f,
    [512:768)=g, [768:1024)=o; partition 32q+b <-> (batch b, H-col 256q+n).
    So i/f/g/o/c are all partition-aligned [128, 256] tiles (per-lane
    engines cannot cross partitions).
  - x_t*w_in + bias enters as a K=2 matmul (rows {x_t, ones} x {w_in, bias})
    with start=True; the 8 K-tiles of h @ W_hh^T accumulate on top.
  - h_new [128, 256] is re-transposed to hT (h^T, K-tile-major with column
    order HT_ORDER) by 4 PE transposes of [64,128] blocks for the next
    step's stationary operand.
  - Final FC: per-partition dot + reduce; the cross-partition 4-way add is
    done exactly via a DRAM round-trip rearrange [128,1] -> [32,4].
"""

import numpy as np
import ml_dtypes

import concourse.bass as bass
import concourse.mybir as mybir
from concourse import bacc
from concourse.tile import TileContext
from concourse.bass_utils import run_bass_kernel_spmd

B, T, H = 256, 512, 1024
NCORES = 8
BL = B // NCORES  # 32 batch rows per core
KT = H // 128  # 8 contraction tiles
F32 = mybir.dt.float32
BF16 = mybir.dt.bfloat16
NPBF16 = ml_dtypes.bfloat16

X1_CHUNK = 64  # steps per x1 SBUF prefetch chunk

# hT column layout: K-tiles stored in order [0,2,4,6,1,3,5,7] (32 cols each).
# Full [128,128] transpose of h_new cols [128c:128c+128] yields tiles
# {2q+c for q in 0..3} as its four 32-col groups.
HT_ORDER = [0, 2, 4, 6, 1, 3, 5, 7]
HT_COL = {k: 32 * HT_ORDER.index(k) for k in range(8)}

# bf16 packed blob column offsets
PKB_WT = 0
PKB_XB = PKB_WT + KT * 4096
PKB_HT0 = PKB_XB + 4096
PKB_IDN = PKB_HT0 + KT * BL
PKB_COLS = PKB_IDN + 128

# f32 packed blob column offsets
PKF_C0 = 0
PKF_WFC = PKF_C0 + 256
PKF_BFC = PKF_WFC + 256
PKF_COLS = PKF_BFC + 1


def build_nc(n_steps: int = T, repeat: int = 1, dbg_skip_elem: bool = False) -> bass.Bass:
    nc = bacc.Bacc()

    initb_d = nc.declare_dram_parameter("initb", [128, PKB_COLS], BF16, isOutput=False)
    initf_d = nc.declare_dram_parameter("initf", [128, PKF_COLS], F32, isOutput=False)
    x1_d = nc.declare_dram_parameter("x1", [2, n_steps * BL], BF16, isOutput=False)
    out_d = nc.declare_dram_parameter("out", [BL, 1], F32, isOutput=True)
    scr_d = nc.dram_tensor("scratch", [128], F32)

    with TileContext(nc) as tc:
        with (
            tc.tile_pool(name="consts", bufs=1) as consts,
            tc.tile_pool(name="state", bufs=1) as state,
            tc.tile_pool(name="x1pool", bufs=2) as x1pool,
            tc.tile_pool(name="work", bufs=2) as work,
            tc.tile_pool(name="psum", bufs=3, space="PSUM") as psum,
            tc.tile_pool(name="psumt", bufs=2, space="PSUM") as psumt,
        ):
            pkb = consts.tile([128, PKB_COLS], BF16)
            nc.sync.dma_start(out=pkb, in_=initb_d[:, :])
            pkf = consts.tile([128, PKF_COLS], F32)
            nc.sync.dma_start(out=pkf, in_=initf_d[:, :])
            wt_sb = pkb[:, PKB_WT : PKB_WT + KT * 4096]
            xb_sb = pkb[0:2, PKB_XB : PKB_XB + 4096]
            idn128 = pkb[:, PKB_IDN : PKB_IDN + 128]
            wfc_sb = pkf[:, PKF_WFC : PKF_WFC + 256]
            bfc_sb = pkf[0:BL, PKF_BFC : PKF_BFC + 1]

            # Repeat loop (timing harness: re-runs the whole recurrence).
            rep_ctx = tc.For_i(0, repeat, 1) if repeat > 1 else None
            if rep_ctx is not None:
                rep_ctx.__enter__()

            # Mutable state: copied out of the packed blobs on-chip.
            hT = state.tile([128, KT * BL], BF16)
            nc.vector.tensor_copy(hT, pkb[:, PKB_HT0 : PKB_HT0 + KT * BL])
            c_sb = state.tile([128, 256], F32)
            nc.vector.tensor_copy(c_sb, pkf[:, PKF_C0 : PKF_C0 + 256])

            x1c = None
            hnew = None

            def emit_k2(xsl, psA, psB):
                for half, pst in ((0, psA), (1, psB)):
                    for q in range(4):
                        nc.tensor.matmul(
                            pst[32 * q : 32 * q + 32, :],
                            xsl,
                            xb_sb[
                                :, 1024 * q + 512 * half : 1024 * q + 512 * half + 512
                            ],
                            start=True,
                            stop=False,
                            tile_position=(0, 32 * q),
                            skip_group_check=True,
                        )

            def emit_main_round(k, half, pst):
                lt = hT[:, HT_COL[k] : HT_COL[k] + BL]
                for q in range(4):
                    base = 4096 * k + 1024 * q + 512 * half
                    nc.tensor.matmul(
                        pst[32 * q : 32 * q + 32, :],
                        lt,
                        wt_sb[:, base : base + 512],
                        start=False,
                        stop=(k == KT - 1),
                        tile_position=(0, 32 * q),
                        skip_group_check=True,
                    )

            x1c = x1pool.tile([2, X1_CHUNK * BL], BF16, name="x1c")
            nc.sync.dma_start(
                out=x1c[:, : min(X1_CHUNK, n_steps) * BL],
                in_=x1_d[:, : min(X1_CHUNK, n_steps) * BL],
            )
            psA = psum.tile([128, 512], F32, name="psA")
            psB = psum.tile([128, 512], F32, name="psB")
            emit_k2(x1c[:, 0:BL], psA, psB)

            for t in range(n_steps):
                # Main matmuls: interleave halves for k<6 so both PSUMs fill
                # together, then finish psA early (its sigmoid overlaps the
                # trailing psB rounds).
                for k in range(KT - 2):
                    emit_main_round(k, 0, psA)
                    emit_main_round(k, 1, psB)
                for k in range(KT - 2, KT):
                    emit_main_round(k, 0, psA)
                pA, pB = psA, psB

                # next step's input-side matmul (independent of h) keeps PE
                # busy during this step's elementwise tail
                if t + 1 < n_steps:
                    u1 = (t + 1) % X1_CHUNK
                    if u1 == 0:
                        nst = min(X1_CHUNK, n_steps - (t + 1))
                        x1c = x1pool.tile([2, X1_CHUNK * BL], BF16, name="x1c")
                        nc.sync.dma_start(
                            out=x1c[:, : nst * BL],
                            in_=x1_d[:, (t + 1) * BL : (t + 1 + nst) * BL],
                        )
                    psA = psum.tile([128, 512], F32, name="psA")
                    psB = psum.tile([128, 512], F32, name="psB")
                    emit_k2(x1c[:, u1 * BL : (u1 + 1) * BL], psA, psB)

                for k in range(KT - 2, KT):
                    emit_main_round(k, 1, pB)

                if dbg_skip_elem:
                    if t != n_steps - 1:
                        psT = psumt.tile([128, KT * BL], BF16, name="psT")
                        for c in range(2):
                            nc.tensor.matmul(
                                psT[:, 128 * c : 128 * c + 128],
                                pkb[:, 128 * c : 128 * c + 128],
                                idn128,
                                is_transpose=True,
                                start=True,
                                stop=True,
                                skip_group_check=True,
                            )
                        nc.vector.tensor_copy(hT, psT)
                    continue

                # Elementwise. pA cols: [0:256)=i [256:512)=f; pB: g, o.
                # tanh(x) = 2*sigmoid(2x)-1; the 2x is pre-folded into the
                # g-gate weights, and h is kept as h/2 (the 2x folded into
                # W_hh/W_fc columns), so tanh affines collapse into
                # scalar_tensor_tensor ops and the ACT engine runs plain
                # back-to-back sigmoids.
                sif = work.tile([128, 512], F32, name="sif")
                nc.scalar.activation(
                    sif, pA, mybir.ActivationFunctionType.Sigmoid
                )
                t1 = work.tile([128, 256], F32, name="t1")
                nc.vector.tensor_mul(t1, sif[:, 256:512], c_sb)
                sgo = work.tile([128, 512], F32, name="sgo")
                nc.scalar.activation(
                    sgo, pB, mybir.ActivationFunctionType.Sigmoid
                )
                u = work.tile([128, 256], F32, name="u")
                nc.vector.scalar_tensor_tensor(
                    out=u, in0=sgo[:, 0:256], scalar=-0.5, in1=sif[:, 0:256],
                    op0=mybir.AluOpType.add, op1=mybir.AluOpType.mult,
                )
                nc.vector.scalar_tensor_tensor(
                    out=c_sb, in0=u, scalar=2.0, in1=t1,
                    op0=mybir.AluOpType.mult, op1=mybir.AluOpType.add,
                )
                sc = work.tile([128, 256], F32, name="sc")
                nc.scalar.activation(
                    sc, c_sb, mybir.ActivationFunctionType.Sigmoid, scale=2.0
                )
                hnew = work.tile([128, 256], BF16, name="hnew")
                nc.vector.scalar_tensor_tensor(
                    out=hnew, in0=sc, scalar=-0.5, in1=sgo[:, 256:512],
                    op0=mybir.AluOpType.add, op1=mybir.AluOpType.mult,
                )

                # Transpose h_new -> hT for next step: 2 full [128,128]
                # PE transposes (base partition 0 only; mixing LDW base
                # partitions between transposes wedges the device).
                if t != n_steps - 1:
                    psT = psumt.tile([128, KT * BL], BF16, name="psT")
                    for c in range(2):
                        nc.tensor.matmul(
                            psT[:, 128 * c : 128 * c + 128],
                            hnew[:, 128 * c : 128 * c + 128],
                            idn128,
                            is_transpose=True,
                            start=True,
                            stop=True,
                            skip_group_check=True,
                        )
                    nc.vector.tensor_copy(hT, psT)

            # Final FC: out[b] = sum_H h[b,H]*wfc[H] + b_fc
            if hnew is None:
                hnew = c_sb
            fcm = work.tile([128, 256], F32)
            nc.vector.tensor_mul(fcm, hnew, wfc_sb)
            fcrf = work.tile([128, 1], F32)
            nc.vector.reduce_sum(out=fcrf, in_=fcm, axis=mybir.AxisListType.X)
            # exact cross-partition 4-way add via DRAM round-trip rearrange
            nc.sync.dma_start(out=scr_d[:], in_=fcrf[:, 0])
            fcr4 = work.tile([BL, 4], F32)
            nc.sync.dma_start(
                out=fcr4, in_=scr_d.ap().rearrange("(q b) -> b q", b=BL)
            )
            fco = work.tile([BL, 1], F32)
            nc.vector.reduce_sum(out=fco, in_=fcr4, axis=mybir.AxisListType.X)
            outsb = work.tile([BL, 1], F32)
            nc.vector.tensor_scalar_add(outsb, fco, scalar1=bfc_sb)
            nc.sync.dma_start(out=out_d[:, :], in_=outsb)
            if rep_ctx is not None:
                rep_ctx.__exit__(None, None, None)

    nc.compile()
    return nc


def prep_inputs(y_hist, W_ih, W_hh, b_ih, b_hh, W_fc, b_fc, h0, c0, n_steps: int = T):
    """Build the 8 per-core input maps (host-side numpy re-layouts)."""
    f = np.float32
    W_hh = np.asarray(W_hh, f)
    w_in = np.asarray(W_ih, f)[:, 0]
    bias = (np.asarray(b_ih, f) + np.asarray(b_hh, f)).astype(f)
    W_fc = np.asarray(W_fc, f)
    b_fc = np.asarray(b_fc, f)
    y_hist = np.asarray(y_hist, f)
    h0 = np.asarray(h0, f)
    c0 = np.asarray(c0, f)

    # wt[p, 4096k + 1024q + 256gi + n] = W_hh[1024gi + 256q + n, 128k + p]
    # gi order per (k,q): 0=i 1=f 2=g 3=o. Scales folded in:
    #  - g-gate outputs pre-doubled (tanh via sigmoid(2x)),
    #  - h stored as h/2 -> all wt entries doubled, wfc doubled, ht0 halved.
    wt = np.zeros((128, KT * 4096), f)
    xb = np.zeros((2, 4096), f)
    for q in range(4):
        for gi in range(4):
            gs = 2.0 if gi == 2 else 1.0
            src = slice(1024 * gi + 256 * q, 1024 * gi + 256 * q + 256)
            for k in range(KT):
                dst = slice(
                    4096 * k + 1024 * q + 256 * gi,
                    4096 * k + 1024 * q + 256 * gi + 256,
                )
                wt[:, dst] = (2.0 * gs) * W_hh[src, 128 * k : 128 * (k + 1)].T
            xb[0, 1024 * q + 256 * gi : 1024 * q + 256 * gi + 256] = gs * w_in[src]
            xb[1, 1024 * q + 256 * gi : 1024 * q + 256 * gi + 256] = gs * bias[src]

    wfc = 2.0 * np.vstack(
        [np.tile(W_fc[0, 256 * q : 256 * (q + 1)], (32, 1)) for q in range(4)]
    ).astype(f)
    bfc = float(np.asarray(b_fc).reshape(-1)[0])
    idn128 = np.eye(128, dtype=f)

    in_maps = []
    for i in range(NCORES):
        b0 = BL * i
        ys = y_hist[b0 : b0 + BL, :n_steps]  # [32, n_steps]
        x1 = np.stack([ys.T.reshape(-1), np.ones(n_steps * BL, f)])
        h0s = 0.5 * h0[b0 : b0 + BL]
        ht0 = np.concatenate(
            [h0s[:, 128 * k : 128 * (k + 1)].T for k in HT_ORDER], axis=1
        )
        c0s = c0[b0 : b0 + BL]
        c0l = np.vstack([c0s[:, 256 * q : 256 * (q + 1)] for q in range(4)])

        pkb = np.zeros((128, PKB_COLS), NPBF16)
        pkb[:, PKB_WT : PKB_WT + KT * 4096] = wt.astype(NPBF16)
        pkb[0:2, PKB_XB : PKB_XB + 4096] = xb.astype(NPBF16)
        pkb[:, PKB_HT0 : PKB_HT0 + KT * BL] = ht0.astype(NPBF16)
        pkb[:, PKB_IDN : PKB_IDN + 128] = idn128.astype(NPBF16)

        pkf = np.zeros((128, PKF_COLS), f)
        pkf[:, PKF_C0 : PKF_C0 + 256] = c0l
        pkf[:, PKF_WFC : PKF_WFC + 256] = wfc
        pkf[0:BL, PKF_BFC] = bfc

        in_maps.append(
            {
                "initb": np.ascontiguousarray(pkb),
                "initf": np.ascontiguousarray(pkf),
                "x1": np.ascontiguousarray(x1.astype(NPBF16)),
            }
        )
    return in_maps


def run(inputs: dict, n_steps: int = T, trace: bool = False):
    nc = build_nc(n_steps)
    in_maps = prep_inputs(**inputs, n_steps=n_steps)
    res = run_bass_kernel_spmd(nc, in_maps, list(range(NCORES)), trace=trace)
    out = np.concatenate([res.results[i]["out"] for i in range(NCORES)], axis=0)
    return out, res


def kernel(**inputs) -> np.ndarray:
    out, _ = run(inputs, n_steps=T)
    return out



# revision 6
# speedup vs baseline: 1.5978x; 1.5978x over previous
"""Trainium2 Bass kernel for nn_Decoder (LSTM over T steps + final FC), v2.

Problem: y_hist [256, 512], LSTM(input_size=1, hidden=1024), h0/c0 [256, 1024],
output = h_T @ W_fc.T + b_fc -> [256, 1].

Sharding: data-parallel. Batch 256 -> 8 cores x 32 rows. Weights replicated;
the time recurrence stays local per core (no collectives).

v2 structure (vs the v1 serial-tail design):
  - Single PE tiling mode (128x32 col-tiled) for EVERY PE instruction:
    * main gate waves: 4 concurrent M=32 MMs (tile_position (0,32q)), moving
      N=512 slices of W_hh^T;
    * the x_t*w_in + bias seed enters as a K=128 matmul whose stationary is
      a zero-padded [128, 32] x-slice (rows 0: x_t, 1: ones, 2..127: zeros)
      and whose moving blob has rows 2..127 zero -- no (32,32)-mode switch;
    * transposes h_new -> hT are 4 col-tiled MMs against an identity moving
      operand (psT[32s:32s+32,:] = hnew[:,32s:32s+32].T @ I) -- no
      transpose-mode switch.  Mode switches would drain the PE array.
  - Gates are computed in two 128-column chunks per 256-col H-block
    (psC0 = [i|f|g|o] for H%256 in [0,128), psC1 for [128,256)), each in its
    own PSUM bank.  Chunk-c elementwise (sigmoids on ACT, combines on DVE,
    the (g-0.5)*i term on GpSimd) runs while the other chunk's waves stream,
    and next step's waves are ordered so each hT half is consumed as late as
    its producing chain allows (chunk0 -> even K-tiles, chunk1 -> odd).
  - ACT is the only engine parked polling the PE semaphore during wave
    bursts (a second parked poller measurably slows MM completion).
  - x/ones stationaries for all T steps are preloaded once (no in-loop DMA).
  - tanh via sigmoid: g-gate weights pre-doubled, h stored as h/2 (W and
    W_fc columns doubled), so the whole elementwise runs on plain sigmoids
    plus scalar_tensor_tensor ops.  c state stays fp32.
"""

import numpy as np
import ml_dtypes

import concourse.bass as bass
import concourse.mybir as mybir
from concourse import bacc
from concourse.tile import TileContext
from concourse.bass_utils import run_bass_kernel_spmd

B, T, H = 256, 512, 1024
NCORES = 8
BL = B // NCORES  # 32 batch rows per core
KT = H // 128  # 8 contraction tiles
F32 = mybir.dt.float32
BF16 = mybir.dt.bfloat16
NPBF16 = ml_dtypes.bfloat16

# bf16 packed blob column offsets: wt | xb | idn | ht0
PKB_WT = 0
PKB_XB = PKB_WT + KT * 4096
PKB_IDN = PKB_XB + 4096
PKB_HT0 = PKB_IDN + 128
PKB_COLS = PKB_HT0 + 256

# f32 packed blob column offsets
PKF_C0 = 0
PKF_WFC = PKF_C0 + 256
PKF_BFC = PKF_WFC + 256
PKF_COLS = PKF_BFC + 1


def ht_col(k: int) -> int:
    # hT layout: even tiles at cols 0..127 (order 0,2,4,6), odd at 128..255
    return 128 * (k % 2) + 32 * (k // 2)


def build_nc(n_steps: int = T) -> bass.Bass:
    nc = bacc.Bacc()

    initb_d = nc.declare_dram_parameter("initb", [128, PKB_COLS], BF16, isOutput=False)
    initf_d = nc.declare_dram_parameter("initf", [128, PKF_COLS], F32, isOutput=False)
    x1_d = nc.declare_dram_parameter("x1", [128, n_steps * BL], BF16, isOutput=False)
    out_d = nc.declare_dram_parameter("out", [BL, 1], F32, isOutput=True)
    scr_d = nc.dram_tensor("scratch", [128], F32)

    with TileContext(nc) as tc:
        with (
            tc.tile_pool(name="consts", bufs=1) as consts,
            tc.tile_pool(name="state", bufs=1) as state,
            tc.tile_pool(name="work", bufs=2) as work,
            tc.tile_pool(name="psum", bufs=2, space="PSUM") as psum,
            tc.tile_pool(name="psumt", bufs=2, space="PSUM") as psumt,
        ):
            pkb = consts.tile([128, PKB_COLS], BF16)
            nc.sync.dma_start(out=pkb, in_=initb_d[:, :])
            pkf = consts.tile([128, PKF_COLS], F32)
            nc.sync.dma_start(out=pkf, in_=initf_d[:, :])
            x1 = consts.tile([128, n_steps * BL], BF16)
            nc.sync.dma_start(out=x1, in_=x1_d[:, :])
            wt_sb = pkb[:, PKB_WT : PKB_WT + KT * 4096]
            xb_sb = pkb[:, PKB_XB : PKB_XB + 4096]
            idn128 = pkb[:, PKB_IDN : PKB_IDN + 128]
            wfc_sb = pkf[:, PKF_WFC : PKF_WFC + 256]
            bfc_sb = pkf[0:BL, PKF_BFC : PKF_BFC + 1]

            # state: two hT buffers (read t%2, write (t+1)%2), fp32 c
            hT2 = [state.tile([128, 256], BF16, name=f"hT{i}") for i in range(2)]
            nc.vector.tensor_copy(hT2[0], pkb[:, PKB_HT0 : PKB_HT0 + 256])
            c_sb = state.tile([128, 256], F32)
            nc.vector.tensor_copy(c_sb, pkf[:, PKF_C0 : PKF_C0 + 256])

            def k2_wave(psC, t, c):
                xs = x1[:, BL * t : BL * t + BL]
                for q in range(4):
                    nc.tensor.matmul(
                        psC[32 * q : 32 * q + 32, :],
                        xs,
                        xb_sb[:, 2048 * c + 512 * q : 2048 * c + 512 * q + 512],
                        start=True,
                        stop=False,
                        tile_position=(0, 32 * q),
                        skip_group_check=True,
                    )

            def main_wave(psC, hT_cur, k, c, stop):
                lt = hT_cur[:, ht_col(k) : ht_col(k) + BL]
                base = 4096 * k + 2048 * c
                for q in range(4):
                    nc.tensor.matmul(
                        psC[32 * q : 32 * q + 32, :],
                        lt,
                        wt_sb[:, base + 512 * q : base + 512 * q + 512],
                        start=False,
                        stop=stop,
                        tile_position=(0, 32 * q),
                        skip_group_check=True,
                    )

            def tr_wave(psT, hn):
                for s in range(4):
                    nc.tensor.matmul(
                        psT[32 * s : 32 * s + 32, :],
                        hn[:, 32 * s : 32 * s + 32],
                        idn128,
                        start=True,
                        stop=True,
                        tile_position=(0, 32 * s),
                        skip_group_check=True,
                    )

            def elementwise(psC_c, c, hn_name):
                sg = work.tile([128, 384], F32, name=f"sg{c}")
                nc.scalar.activation(
                    sg, psC_c[:, 0:384], mybir.ActivationFunctionType.Sigmoid
                )
                sgo = work.tile([128, 128], F32, name=f"sgo{c}")
                nc.scalar.activation(
                    sgo, psC_c[:, 384:512], mybir.ActivationFunctionType.Sigmoid
                )
                cch = c_sb[:, 128 * c : 128 * c + 128]
                t1 = work.tile([128, 128], F32, name=f"t1{c}")
                nc.vector.tensor_mul(t1, sg[:, 128:256], cch)
                u = work.tile([128, 128], F32, name=f"u{c}")
                nc.vector.scalar_tensor_tensor(
                    out=u, in0=sg[:, 256:384], scalar=-0.5, in1=sg[:, 0:128],
                    op0=mybir.AluOpType.add, op1=mybir.AluOpType.mult,
                )
                nc.vector.scalar_tensor_tensor(
                    out=cch, in0=u, scalar=2.0, in1=t1,
                    op0=mybir.AluOpType.mult, op1=mybir.AluOpType.add,
                )
                sc = work.tile([128, 128], F32, name=f"sc{c}")
                nc.scalar.activation(
                    sc, cch, mybir.ActivationFunctionType.Sigmoid, scale=2.0
                )
                hn = work.tile([128, 128], BF16, name=hn_name)
                nc.vector.scalar_tensor_tensor(
                    out=hn, in0=sc, scalar=-0.5, in1=sgo,
                    op0=mybir.AluOpType.add, op1=mybir.AluOpType.mult,
                )
                return hn

            # Per-cycle emission (cycle t):
            #   PE:  K2a K2b | ev0 | TRw1(t-1) | ev1[0] od0 ev1[1:] od1 | TRw0(t)
            #   DVE: cp1(t-1) | t1_0 c_0 hn_0 cp0(t) | t1_1 c_1 hn_1
            #   ACT: sg0 sgo0 sc0 | sg1 sgo1 sc1     GP: u_0 | u_1
            # TRw1(t) is deferred into cycle t+1 (after ev0) so the PE never
            # stalls on chunk-1's elementwise chain; each hT half is copied
            # right after its transpose completes.
            hn_prev1 = None  # hnew1(t-1) awaiting its deferred transpose
            hn_cur = [None, None]
            for t in range(n_steps):
                hT_cur = hT2[t % 2]
                hT_nxt = hT2[(t + 1) % 2]
                psC = [
                    psum.tile([128, 512], F32, name="psC0"),
                    psum.tile([128, 512], F32, name="psC1"),
                ]

                k2_wave(psC[0], t, 0)
                k2_wave(psC[1], t, 1)
                for k in (0, 2, 4, 6):  # ev0
                    main_wave(psC[0], hT_cur, k, 0, False)
                if hn_prev1 is not None:
                    psT1 = psumt.tile([128, 128], F32, name="psT1")
                    tr_wave(psT1, hn_prev1)
                    nc.vector.tensor_copy(hT_cur[:, 128:256], psT1)
                main_wave(psC[1], hT_cur, 0, 1, False)  # ev1[0]
                for k in (1, 3, 5, 7):  # od0
                    main_wave(psC[0], hT_cur, k, 0, k == 7)
                for k in (2, 4, 6):  # ev1[1:]
                    main_wave(psC[1], hT_cur, k, 1, False)
                for k in (1, 3, 5, 7):  # od1
                    main_wave(psC[1], hT_cur, k, 1, k == 7)

                hn_cur = [None, None]
                hn_cur[0] = elementwise(psC[0], 0, "hn0")
                if t + 1 < n_steps:
                    psT0 = psumt.tile([128, 128], F32, name="psT0")
                    tr_wave(psT0, hn_cur[0])
                    nc.vector.tensor_copy(hT_nxt[:, 0:128], psT0)
                hn_cur[1] = elementwise(psC[1], 1, "hn1")
                hn_prev1 = hn_cur[1] if t + 1 < n_steps else None
            # Final FC: out[b] = sum_H h[b,H]*wfc[H] + b_fc
            fcm = work.tile([128, 256], F32)
            nc.vector.tensor_mul(fcm[:, 0:128], hn_cur[0], wfc_sb[:, 0:128])
            nc.vector.tensor_mul(fcm[:, 128:256], hn_cur[1], wfc_sb[:, 128:256])
            fcrf = work.tile([128, 1], F32)
            nc.vector.reduce_sum(out=fcrf, in_=fcm, axis=mybir.AxisListType.X)
            # exact cross-partition 4-way add via DRAM round-trip rearrange
            nc.sync.dma_start(out=scr_d[:], in_=fcrf[:, 0])
            fcr4 = work.tile([BL, 4], F32)
            nc.sync.dma_start(
                out=fcr4, in_=scr_d.ap().rearrange("(q b) -> b q", b=BL)
            )
            fco = work.tile([BL, 1], F32)
            nc.vector.reduce_sum(out=fco, in_=fcr4, axis=mybir.AxisListType.X)
            outsb = work.tile([BL, 1], F32)
            nc.vector.tensor_scalar_add(outsb, fco, scalar1=bfc_sb)
            nc.sync.dma_start(out=out_d[:, :], in_=outsb)

    nc.compile()
    return nc


def prep_inputs(y_hist, W_ih, W_hh, b_ih, b_hh, W_fc, b_fc, h0, c0, n_steps: int = T):
    """Build the 8 per-core input maps (host-side numpy re-layouts)."""
    f = np.float32
    W_hh = np.asarray(W_hh, f)
    w_in = np.asarray(W_ih, f)[:, 0]
    bias = (np.asarray(b_ih, f) + np.asarray(b_hh, f)).astype(f)
    W_fc = np.asarray(W_fc, f)
    b_fc = np.asarray(b_fc, f)
    y_hist = np.asarray(y_hist, f)
    h0 = np.asarray(h0, f)
    c0 = np.asarray(c0, f)

    # wt[p, 4096k + 2048c + 512q + 128gi + j] = 2*gs*W_hh[1024gi+256q+128c+j, 128k+p]
    # gi order: 0=i 1=f 2=g 3=o; gs=2 for the g gate (tanh via sigmoid(2x));
    # global 2x because h is stored as h/2.
    wt = np.zeros((128, KT * 4096), f)
    xb = np.zeros((128, 4096), f)
    for c in range(2):
        for q in range(4):
            for gi in range(4):
                gs = 2.0 if gi == 2 else 1.0
                src = slice(
                    1024 * gi + 256 * q + 128 * c,
                    1024 * gi + 256 * q + 128 * c + 128,
                )
                for k in range(KT):
                    dst = slice(
                        4096 * k + 2048 * c + 512 * q + 128 * gi,
                        4096 * k + 2048 * c + 512 * q + 128 * gi + 128,
                    )
                    wt[:, dst] = (2.0 * gs) * W_hh[src, 128 * k : 128 * (k + 1)].T
                dstx = slice(
                    2048 * c + 512 * q + 128 * gi,
                    2048 * c + 512 * q + 128 * gi + 128,
                )
                xb[0, dstx] = gs * w_in[src]
                xb[1, dstx] = gs * bias[src]

    wfc = 2.0 * np.vstack(
        [np.tile(W_fc[0, 256 * q : 256 * (q + 1)], (BL, 1)) for q in range(4)]
    ).astype(f)
    bfc = float(np.asarray(b_fc).reshape(-1)[0])
    idn128 = np.eye(128, dtype=f)

    in_maps = []
    for i in range(NCORES):
        b0 = BL * i
        ys = y_hist[b0 : b0 + BL, :n_steps]  # [32, n_steps]
        x1 = np.zeros((128, n_steps * BL), f)
        x1[0, :] = ys.T.reshape(-1)
        x1[1, :] = 1.0
        h0s = 0.5 * h0[b0 : b0 + BL]
        ht0 = np.zeros((128, 256), f)
        for k in range(KT):
            ht0[:, ht_col(k) : ht_col(k) + BL] = h0s[:, 128 * k : 128 * (k + 1)].T
        c0s = c0[b0 : b0 + BL]
        c0l = np.vstack([c0s[:, 256 * q : 256 * (q + 1)] for q in range(4)])

        pkb = np.zeros((128, PKB_COLS), NPBF16)
        pkb[:, PKB_WT : PKB_WT + KT * 4096] = wt.astype(NPBF16)
        pkb[:, PKB_XB : PKB_XB + 4096] = xb.astype(NPBF16)
        pkb[:, PKB_IDN : PKB_IDN + 128] = idn128.astype(NPBF16)
        pkb[:, PKB_HT0 : PKB_HT0 + 256] = ht0.astype(NPBF16)

        pkf = np.zeros((128, PKF_COLS), f)
        pkf[:, PKF_C0 : PKF_C0 + 256] = c0l
        pkf[:, PKF_WFC : PKF_WFC + 256] = wfc
        pkf[0:BL, PKF_BFC] = bfc

        in_maps.append(
            {
                "initb": np.ascontiguousarray(pkb),
                "initf": np.ascontiguousarray(pkf),
                "x1": np.ascontiguousarray(x1.astype(NPBF16)),
            }
        )
    return in_maps


def run(inputs: dict, n_steps: int = T, trace: bool = False):
    nc = build_nc(n_steps)
    in_maps = prep_inputs(**inputs, n_steps=n_steps)
    res = run_bass_kernel_spmd(nc, in_maps, list(range(NCORES)), trace=trace)
    out = np.concatenate([res.results[i]["out"] for i in range(NCORES)], axis=0)
    return out, res


def kernel(**inputs) -> np.ndarray:
    out, _ = run(inputs, n_steps=T)
    return out
